# revision 8
# baseline (speedup 1.0000x reference)
"""Multi-head attention (B=2, S=2048, D=1024, H=16, d_k=64) on 8 TRN2 NeuronCores.

Sharding: batch x head-groups. Core c handles batch b = c // 4 and heads
[4*(c%4), 4*(c%4)+4), i.e. a 256-wide slice of the model dim. Each core:
  - casts its batch's q/k/v activations to bf16 and transposes them on-chip
    (DMA xbar) to get the model dim onto partitions,
  - projects to Q^T/K^T (head-dims on partitions) and V (tokens on partitions),
  - computes transposed scores S^T = K Q^T per head (keys on partitions),
    exp via ScalarE (softmax max-subtraction is unnecessary at these scales:
    scores ~ N(0,1)), and attention output via [V | 1] augmented matmuls that
    also produce the softmax denominators,
  - normalizes, applies the output projection against a 256-row slice of Wo,
    and writes a partial y to HBM.
Host sums the 4 partial y's per batch and adds bo.

Matmuls run as float32r (PE full rate for free dim >= 256, ~1.6e-4 rel err)
except the input projections, which are bf16 (the DMA-transpose path is
2-byte only).
"""

import numpy as np

B, S, D = 2, 2048, 1024
H, DK = 16, 64
NCORES = 8
DS = 256            # model-dim slice per core (4 heads x 64)
P = 128

_cache = {}


def _build():
    import concourse.bass as bass
    import concourse.mybir as mybir
    import concourse.tile as tile
    from concourse import bacc

    f32 = mybir.dt.float32
    f32r = mybir.dt.float32r
    bf16 = mybir.dt.bfloat16
    Exp = mybir.ActivationFunctionType.Exp
    add = mybir.AluOpType.add
    div = mybir.AluOpType.divide

    nc = bacc.Bacc("TRN2", target_bir_lowering=False, debug=False,
                   num_devices=NCORES)

    xq_d = nc.dram_tensor("xq", [S, D], f32, kind="ExternalInput")
    xk_d = nc.dram_tensor("xk", [S, D], f32, kind="ExternalInput")
    xv_d = nc.dram_tensor("xv", [S, D], f32, kind="ExternalInput")
    wqT_d = nc.dram_tensor("wqT", [D, DS], f32, kind="ExternalInput")
    wkT_d = nc.dram_tensor("wkT", [D, DS], f32, kind="ExternalInput")
    wvT_d = nc.dram_tensor("wvT", [D, DS], f32, kind="ExternalInput")
    woT_d = nc.dram_tensor("woT", [DS, D], f32, kind="ExternalInput")
    bq_d = nc.dram_tensor("bq", [2, P, 1], f32, kind="ExternalInput")
    bk_d = nc.dram_tensor("bk", [2, P, 1], f32, kind="ExternalInput")
    bv_d = nc.dram_tensor("bv", [1, DS], f32, kind="ExternalInput")
    y_d = nc.dram_tensor("y", [S, D], f32, kind="ExternalOutput")

    with tile.TileContext(nc) as tc:
        with (
            tc.tile_pool(name="persist", bufs=1) as pp,
            tc.tile_pool(name="xload", bufs=4) as xp,
            tc.tile_pool(name="xT", bufs=1) as xtp,
            tc.tile_pool(name="pt", bufs=6) as ptp,
            tc.tile_pool(name="small", bufs=4) as smp,
            tc.tile_pool(name="ysb", bufs=2) as yp,
        ):
            # ---- constants / weights ----
            wq_bf = pp.tile([P, 8, DS], bf16)
            wk_bf = pp.tile([P, 8, DS], bf16)
            wv_bf = pp.tile([P, 8, DS], bf16)
            for w_bf, w_d in ((wq_bf, wqT_d), (wk_bf, wkT_d), (wv_bf, wvT_d)):
                for c in range(8):
                    nc.gpsimd.dma_start(w_bf[:, c, :],
                                        w_d.ap()[c * P:(c + 1) * P, :])
            wo_bf = pp.tile([P, 2, D], bf16)
            for c in range(2):
                nc.gpsimd.dma_start(wo_bf[:, c, :], woT_d.ap()[c * P:(c + 1) * P, :])

            bq_sb = pp.tile([P, 2, 1], f32)
            bk_sb = pp.tile([P, 2, 1], f32)
            for hp in range(2):
                nc.sync.dma_start(bq_sb[:, hp, :], bq_d.ap()[hp])
                nc.sync.dma_start(bk_sb[:, hp, :], bk_d.ap()[hp])
            bv_bf = pp.tile([1, DS], bf16)
            nc.gpsimd.dma_start(bv_bf[:], bv_d.ap())

            ones_bf = pp.tile([1, P], bf16)
            nc.vector.memset(ones_bf[:], 1.0)
            ones32 = pp.tile([P, 64], f32)
            nc.vector.memset(ones32[:], 1.0)
            ones_r = pp.tile([1, 64], f32r)
            nc.vector.tensor_copy(ones_r[:], ones32[0:1, :])

            # ---- persistent activations ----
            QT = pp.tile([P, 2, S], f32r)      # [dim-in-pair, head-pair, token]
            KT = pp.tile([P, 2, S], f32r)
            V = pp.tile([P, 16, 4 * 65], f32r)  # [token-in-tile, token-tile, head*65]
            attnT = pp.tile([P, 2, S], bf16)   # normalized attention out^T

            # V ones column (softmax denominator generator): col 64 of each head
            v4 = V[:].rearrange("p t (h c) -> p t h c", h=4)
            nc.vector.tensor_copy(v4[:, :, :, 64:65], ones32[:, 0:64])

            # ---- phase 1: load + transpose + project ----
            pj_ctx = tc.tile_pool(name="pj_ps", bufs=4, space="PSUM")
            pjp = pj_ctx.__enter__()
            for kind, x_d, w_bf in (("q", xq_d, wq_bf), ("k", xk_d, wk_bf),
                                    ("v", xv_d, wv_bf)):
                xT = xtp.tile([P, 8, S], bf16, tag="xT")
                for tb in range(16):
                    xn = xp.tile([P, D], bf16, tag="xn")
                    nc.gpsimd.dma_start(xn[:], x_d.ap()[tb * P:(tb + 1) * P, :])
                    nc.sync.dma_start_transpose(xT[:, :, tb * P:(tb + 1) * P],
                                                xn[:])
                if kind in ("q", "k"):
                    out_t = QT if kind == "q" else KT
                    bias = bq_sb if kind == "q" else bk_sb
                    for hp in range(2):
                        for t4 in range(4):
                            ps = pjp.tile([P, 512], f32, tag="pj")
                            for ch in range(8):
                                nc.tensor.matmul(
                                    ps[:],
                                    w_bf[:, ch, hp * P:(hp + 1) * P],
                                    xT[:, ch, t4 * 512:(t4 + 1) * 512],
                                    start=(ch == 0), stop=(ch == 7))
                            nc.vector.tensor_scalar(
                                out_t[:, hp, t4 * 512:(t4 + 1) * 512],
                                ps[:], bias[:, hp, :], None, op0=add)
                else:
                    for tb in range(16):
                        ps = pjp.tile([P, DS], f32, tag="pv")
                        for ch in range(8):
                            nc.tensor.matmul(
                                ps[:],
                                xT[:, ch, tb * P:(tb + 1) * P],
                                w_bf[:, ch, :],
                                start=(ch == 0), stop=False)
                        nc.tensor.matmul(ps[:], ones_bf[:], bv_bf[:],
                                         start=False, stop=True)
                        pv4 = ps[:].rearrange("p (h c) -> p h c", h=4)
                        nc.vector.tensor_copy(v4[:, tb, :, 0:64], pv4[:])

            pj_ctx.__exit__(None, None, None)

            # ---- phase 2+3: attention per (qb), heads pairwise; then y ----
            sc_ctx = tc.tile_pool(name="sc_ps", bufs=2, space="PSUM")
            av_ctx = tc.tile_pool(name="av_ps", bufs=1, space="PSUM")
            bc_ctx = tc.tile_pool(name="bc_ps", bufs=2, space="PSUM")
            scp = sc_ctx.__enter__()
            avp = av_ctx.__enter__()
            bcp = bc_ctx.__enter__()
            for qb in range(4):
                qs = slice(qb * 512, (qb + 1) * 512)
                for hp in range(2):
                    av = [avp.tile([65, 512], f32, name=f"av{hh}",
                                   tag=f"av{hh}")
                          for hh in range(2)]
                    for kt in range(16):
                        sct = scp.tile([P, 2, 512], f32, tag="sc")
                        for hh in range(2):
                            hb = 64 * hh
                            nc.tensor.matmul(
                                sct[:, hh, :],
                                KT[hb:hb + 64, hp, kt * P:(kt + 1) * P],
                                QT[hb:hb + 64, hp, qs],
                                start=True, stop=True)
                        pt = ptp.tile([P, 2, 512], f32r, tag="pt")
                        nc.scalar.activation(pt[:], sct[:], Exp, scale=0.125)
                        for hh in range(2):
                            hl = 2 * hp + hh
                            nc.tensor.matmul(
                                av[hh][:],
                                V[:, kt, hl * 65:(hl + 1) * 65],
                                pt[:, hh, :],
                                start=(kt == 0), stop=(kt == 15))
                    # normalize: attnT = av[0:64] * recip(broadcast(av[64]))
                    for hh in range(2):
                        sums_r = smp.tile([1, 512], f32r, tag="sums")
                        nc.vector.tensor_copy(sums_r[:], av[hh][64:65, :])
                        bc = bcp.tile([64, 512], f32, name="bc", tag="bc")
                        nc.tensor.matmul(bc[:], ones_r[:], sums_r[:],
                                         start=True, stop=True)
                        rec_sb = smp.tile([64, 512], f32, name="rec_sb",
                                          tag="bcs")
                        nc.vector.reciprocal(rec_sb[:], bc[:])
                        if hh == 0:
                            nc.vector.tensor_mul(
                                attnT[0:64, hp, qs],
                                av[hh][0:64, :], rec_sb[:])
                        else:
                            a_tmp = smp.tile([64, 512], bf16, name="a_tmp",
                                             tag="atmp")
                            nc.vector.tensor_mul(
                                a_tmp[:], av[hh][0:64, :], rec_sb[:])
                            nc.sync.dma_start(attnT[64:128, hp, qs], a_tmp[:])
                # output projection for this q-range (4 token tiles)
                for tt in range(4 * qb, 4 * qb + 4):
                    y_sb = yp.tile([P, D], f32, tag="y")
                    py = scp.tile([P, 2, 512], f32, name="py", tag="sc")
                    for nb in range(2):
                        for hpc in range(2):
                            nc.tensor.matmul(
                                py[:, nb, :],
                                attnT[:, hpc, tt * P:(tt + 1) * P],
                                wo_bf[:, hpc, nb * 512:(nb + 1) * 512],
                                start=(hpc == 0), stop=(hpc == 1))
                    nc.vector.tensor_copy(y_sb[:], py[:])
                    nc.sync.dma_start(y_d.ap()[tt * P:(tt + 1) * P, :], y_sb[:])

            bc_ctx.__exit__(None, None, None)
            av_ctx.__exit__(None, None, None)
            sc_ctx.__exit__(None, None, None)

    nc.compile()
    return nc


def _shard(query, key, value, Wq, bq, Wk, bk, Wv, bv, Wo, bo):
    f = np.float32
    q = np.ascontiguousarray(query, dtype=f)
    k = np.ascontiguousarray(key, dtype=f)
    v = np.ascontiguousarray(value, dtype=f)
    in_maps = []
    for c in range(NCORES):
        b, hg = c // 4, c % 4
        ds = DS * hg
        in_maps.append({
            "xq": q[b],
            "xk": k[b],
            "xv": v[b],
            "wqT": np.ascontiguousarray(np.asarray(Wq, f)[ds:ds + DS, :].T),
            "wkT": np.ascontiguousarray(np.asarray(Wk, f)[ds:ds + DS, :].T),
            "wvT": np.ascontiguousarray(np.asarray(Wv, f)[ds:ds + DS, :].T),
            "woT": np.ascontiguousarray(np.asarray(Wo, f)[:, ds:ds + DS].T),
            "bq": np.asarray(bq, f)[ds:ds + DS].reshape(2, P, 1),
            "bk": np.asarray(bk, f)[ds:ds + DS].reshape(2, P, 1),
            "bv": np.asarray(bv, f)[ds:ds + DS].reshape(1, DS),
        })
    return in_maps


def _unshard(results, bo):
    y = np.zeros((B, S, D), dtype=np.float64)
    for c in range(NCORES):
        y[c // 4] += results[c]["y"].astype(np.float64)
    y += np.asarray(bo, np.float64)
    return y.astype(np.float32)


def kernel(query, key, value, Wq, bq, Wk, bk, Wv, bv, Wo, bo):
    from concourse.bass_utils import run_bass_kernel_spmd

    if "nc" not in _cache:
        _cache["nc"] = _build()
    nc = _cache["nc"]
    in_maps = _shard(query, key, value, Wq, bq, Wk, bk, Wv, bv, Wo, bo)
    res = run_bass_kernel_spmd(nc, in_maps, core_ids=list(range(NCORES)))
    return _unshard(res.results, bo)


# revision 9
# speedup vs baseline: 1.3575x; 1.3575x over previous
"""Multi-head attention (B=2, S=2048, D=1024, H=16, d_k=64) on 8 TRN2 NeuronCores.

Sharding: batch x head-groups. Core c handles batch b = c // 4 and heads
[4*(c%4), 4*(c%4)+4), i.e. a 256-wide slice of the model dim. Each core:
  - casts its batch's q/k/v activations to bf16 and transposes them on-chip
    (DMA xbar) to get the model dim onto partitions,
  - projects to Q^T/K^T (head-dims on partitions) and V (tokens on partitions),
  - computes transposed scores S^T = K Q^T per head (keys on partitions),
    exp via ScalarE (softmax max-subtraction is unnecessary at these scales:
    scores ~ N(0,1)), and attention output via [V | 1] augmented matmuls that
    also produce the softmax denominators,
  - normalizes, applies the output projection against a 256-row slice of Wo,
    and writes a partial y to HBM.
Host sums the 4 partial y's per batch and adds bo.

Matmuls run as float32r (PE full rate for free dim >= 256, ~1.6e-4 rel err)
except the input projections, which are bf16 (the DMA-transpose path is
2-byte only).
"""

import numpy as np

B, S, D = 2, 2048, 1024
H, DK = 16, 64
NCORES = 8
DS = 256            # model-dim slice per core (4 heads x 64)
P = 128

_cache = {}


def _build():
    import concourse.bass as bass
    import concourse.mybir as mybir
    import concourse.tile as tile
    from concourse import bacc

    f32 = mybir.dt.float32
    f32r = mybir.dt.float32r
    bf16 = mybir.dt.bfloat16
    Exp = mybir.ActivationFunctionType.Exp
    add = mybir.AluOpType.add
    div = mybir.AluOpType.divide

    nc = bacc.Bacc("TRN2", target_bir_lowering=False, debug=False,
                   num_devices=NCORES)

    xq_d = nc.dram_tensor("xq", [S, D], bf16, kind="ExternalInput")
    xk_d = nc.dram_tensor("xk", [S, D], bf16, kind="ExternalInput")
    xv_d = nc.dram_tensor("xv", [S, D], bf16, kind="ExternalInput")
    wqT_d = nc.dram_tensor("wqT", [D, DS], bf16, kind="ExternalInput")
    wkT_d = nc.dram_tensor("wkT", [D, DS], bf16, kind="ExternalInput")
    wvT_d = nc.dram_tensor("wvT", [D, DS], bf16, kind="ExternalInput")
    woT_d = nc.dram_tensor("woT", [DS, D], bf16, kind="ExternalInput")
    bq_d = nc.dram_tensor("bq", [2, P, 1], f32, kind="ExternalInput")
    bk_d = nc.dram_tensor("bk", [2, P, 1], f32, kind="ExternalInput")
    bv_d = nc.dram_tensor("bv", [1, DS], bf16, kind="ExternalInput")
    y_d = nc.dram_tensor("y", [S, D], f32, kind="ExternalOutput")

    with tile.TileContext(nc) as tc:
        with (
            tc.tile_pool(name="persist", bufs=1) as pp,
            tc.tile_pool(name="xT", bufs=1) as xtp,
            tc.tile_pool(name="pt", bufs=6) as ptp,
            tc.tile_pool(name="small", bufs=4) as smp,
            tc.tile_pool(name="ysb", bufs=2) as yp,
        ):
            # ---- constants / weights ----
            wq_bf = pp.tile([P, 8, DS], bf16)
            wk_bf = pp.tile([P, 8, DS], bf16)
            wv_bf = pp.tile([P, 8, DS], bf16)
            for w_bf, w_d in ((wq_bf, wqT_d), (wk_bf, wkT_d), (wv_bf, wvT_d)):
                for c in range(8):
                    nc.sync.dma_start(w_bf[:, c, :],
                                      w_d.ap()[c * P:(c + 1) * P, :])
            wo_bf = pp.tile([P, 2, D], bf16)
            for c in range(2):
                nc.sync.dma_start(wo_bf[:, c, :], woT_d.ap()[c * P:(c + 1) * P, :])

            bq_sb = pp.tile([P, 2, 1], f32)
            bk_sb = pp.tile([P, 2, 1], f32)
            for hp in range(2):
                nc.sync.dma_start(bq_sb[:, hp, :], bq_d.ap()[hp])
                nc.sync.dma_start(bk_sb[:, hp, :], bk_d.ap()[hp])
            bv_bf = pp.tile([1, DS], bf16)
            nc.sync.dma_start(bv_bf[:], bv_d.ap())

            ones_bf = pp.tile([1, P], bf16)
            nc.vector.memset(ones_bf[:], 1.0)
            ones32 = pp.tile([P, 64], f32)
            nc.vector.memset(ones32[:], 1.0)
            ones_r = pp.tile([1, 64], f32r)
            nc.vector.tensor_copy(ones_r[:], ones32[0:1, :])

            # ---- persistent activations ----
            QT = pp.tile([P, 2, S], f32r)      # [dim-in-pair, head-pair, token]
            KT = pp.tile([P, 2, S], f32r)
            V = pp.tile([P, 16, 4 * 65], f32r)  # [token-in-tile, token-tile, head*65]
            attnT = pp.tile([P, 2, S], bf16)   # normalized attention out^T

            # V ones column (softmax denominator generator): col 64 of each head
            v4 = V[:].rearrange("p t (h c) -> p t h c", h=4)
            nc.vector.tensor_copy(v4[:, :, :, 64:65], ones32[:, 0:64])

            # ---- phase 1: load + transpose + project ----
            pj_ctx = tc.tile_pool(name="pj_ps", bufs=4, space="PSUM")
            pjp = pj_ctx.__enter__()
            for kind, x_d, w_bf in (("q", xq_d, wq_bf), ("k", xk_d, wk_bf),
                                    ("v", xv_d, wv_bf)):
                xT = xtp.tile([P, 8, S], bf16, tag="xT")
                for tb in range(16):
                    nc.sync.dma_start_transpose(xT[:, :, tb * P:(tb + 1) * P],
                                                x_d.ap()[tb * P:(tb + 1) * P, :])
                if kind in ("q", "k"):
                    out_t = QT if kind == "q" else KT
                    bias = bq_sb if kind == "q" else bk_sb
                    for hp in range(2):
                        for t4 in range(4):
                            ps = pjp.tile([P, 512], f32, tag="pj")
                            for ch in range(8):
                                nc.tensor.matmul(
                                    ps[:],
                                    w_bf[:, ch, hp * P:(hp + 1) * P],
                                    xT[:, ch, t4 * 512:(t4 + 1) * 512],
                                    start=(ch == 0), stop=(ch == 7))
                            nc.vector.tensor_scalar(
                                out_t[:, hp, t4 * 512:(t4 + 1) * 512],
                                ps[:], bias[:, hp, :], None, op0=add)
                else:
                    for tb in range(16):
                        ps = pjp.tile([P, DS], f32, tag="pv")
                        for ch in range(8):
                            nc.tensor.matmul(
                                ps[:],
                                xT[:, ch, tb * P:(tb + 1) * P],
                                w_bf[:, ch, :],
                                start=(ch == 0), stop=False)
                        nc.tensor.matmul(ps[:], ones_bf[:], bv_bf[:],
                                         start=False, stop=True)
                        pv4 = ps[:].rearrange("p (h c) -> p h c", h=4)
                        nc.vector.tensor_copy(v4[:, tb, :, 0:64], pv4[:])

            pj_ctx.__exit__(None, None, None)

            # ---- phase 2+3: attention per (qb), heads pairwise; then y ----
            sc_ctx = tc.tile_pool(name="sc_ps", bufs=2, space="PSUM")
            av_ctx = tc.tile_pool(name="av_ps", bufs=1, space="PSUM")
            bc_ctx = tc.tile_pool(name="bc_ps", bufs=2, space="PSUM")
            scp = sc_ctx.__enter__()
            avp = av_ctx.__enter__()
            bcp = bc_ctx.__enter__()
            for qb in range(4):
                qs = slice(qb * 512, (qb + 1) * 512)
                for hp in range(2):
                    av = [avp.tile([65, 512], f32, name=f"av{hh}",
                                   tag=f"av{hh}")
                          for hh in range(2)]
                    for kt in range(16):
                        sct = scp.tile([P, 2, 512], f32, tag="sc")
                        for hh in range(2):
                            hb = 64 * hh
                            nc.tensor.matmul(
                                sct[:, hh, :],
                                KT[hb:hb + 64, hp, kt * P:(kt + 1) * P],
                                QT[hb:hb + 64, hp, qs],
                                start=True, stop=True)
                        pt = ptp.tile([P, 2, 512], f32r, tag="pt")
                        nc.scalar.activation(pt[:], sct[:], Exp, scale=0.125)
                        for hh in range(2):
                            hl = 2 * hp + hh
                            nc.tensor.matmul(
                                av[hh][:],
                                V[:, kt, hl * 65:(hl + 1) * 65],
                                pt[:, hh, :],
                                start=(kt == 0), stop=(kt == 15))
                    # normalize: attnT = av[0:64] * recip(broadcast(av[64]))
                    for hh in range(2):
                        sums_r = smp.tile([1, 512], f32r, tag="sums")
                        nc.vector.tensor_copy(sums_r[:], av[hh][64:65, :])
                        bc = bcp.tile([64, 512], f32, name="bc", tag="bc")
                        nc.tensor.matmul(bc[:], ones_r[:], sums_r[:],
                                         start=True, stop=True)
                        rec_sb = smp.tile([64, 512], f32, name="rec_sb",
                                          tag="bcs")
                        nc.vector.reciprocal(rec_sb[:], bc[:])
                        if hh == 0:
                            nc.vector.tensor_mul(
                                attnT[0:64, hp, qs],
                                av[hh][0:64, :], rec_sb[:])
                        else:
                            a_tmp = smp.tile([64, 512], bf16, name="a_tmp",
                                             tag="atmp")
                            nc.vector.tensor_mul(
                                a_tmp[:], av[hh][0:64, :], rec_sb[:])
                            nc.sync.dma_start(attnT[64:128, hp, qs], a_tmp[:])
                # output projection for this q-range (4 token tiles)
                for tt in range(4 * qb, 4 * qb + 4):
                    y_sb = yp.tile([P, D], f32, tag="y")
                    py = scp.tile([P, 2, 512], f32, name="py", tag="sc")
                    for nb in range(2):
                        for hpc in range(2):
                            nc.tensor.matmul(
                                py[:, nb, :],
                                attnT[:, hpc, tt * P:(tt + 1) * P],
                                wo_bf[:, hpc, nb * 512:(nb + 1) * 512],
                                start=(hpc == 0), stop=(hpc == 1))
                    nc.vector.tensor_copy(y_sb[:], py[:])
                    nc.sync.dma_start(y_d.ap()[tt * P:(tt + 1) * P, :], y_sb[:])

            bc_ctx.__exit__(None, None, None)
            av_ctx.__exit__(None, None, None)
            sc_ctx.__exit__(None, None, None)

    nc.compile()
    return nc


def _shard(query, key, value, Wq, bq, Wk, bk, Wv, bv, Wo, bo):
    import ml_dtypes
    f = np.float32
    bf = ml_dtypes.bfloat16
    q = np.asarray(query, dtype=f).astype(bf)
    k = np.asarray(key, dtype=f).astype(bf)
    v = np.asarray(value, dtype=f).astype(bf)
    in_maps = []
    for c in range(NCORES):
        b, hg = c // 4, c % 4
        ds = DS * hg
        in_maps.append({
            "xq": np.ascontiguousarray(q[b]),
            "xk": np.ascontiguousarray(k[b]),
            "xv": np.ascontiguousarray(v[b]),
            "wqT": np.ascontiguousarray(np.asarray(Wq, f)[ds:ds + DS, :].T.astype(bf)),
            "wkT": np.ascontiguousarray(np.asarray(Wk, f)[ds:ds + DS, :].T.astype(bf)),
            "wvT": np.ascontiguousarray(np.asarray(Wv, f)[ds:ds + DS, :].T.astype(bf)),
            "woT": np.ascontiguousarray(np.asarray(Wo, f)[:, ds:ds + DS].T.astype(bf)),
            "bq": np.asarray(bq, f)[ds:ds + DS].reshape(2, P, 1),
            "bk": np.asarray(bk, f)[ds:ds + DS].reshape(2, P, 1),
            "bv": np.asarray(bv, f)[ds:ds + DS].astype(bf).reshape(1, DS),
        })
    return in_maps


def _unshard(results, bo):
    y = np.zeros((B, S, D), dtype=np.float64)
    for c in range(NCORES):
        y[c // 4] += results[c]["y"].astype(np.float64)
    y += np.asarray(bo, np.float64)
    return y.astype(np.float32)


def kernel(query, key, value, Wq, bq, Wk, bk, Wv, bv, Wo, bo):
    from concourse.bass_utils import run_bass_kernel_spmd

    if "nc" not in _cache:
        _cache["nc"] = _build()
    nc = _cache["nc"]
    in_maps = _shard(query, key, value, Wq, bq, Wk, bk, Wv, bv, Wo, bo)
    res = run_bass_kernel_spmd(nc, in_maps, core_ids=list(range(NCORES)))
    return _unshard(res.results, bo)
